# revision 31
# baseline (speedup 1.0000x reference)
"""Multi-head attention (B=4, S=2048, D=1024, H=16) on 8 Trainium2 NeuronCores.

Sharding: tensor-parallel over heads. Core c owns heads 2c, 2c+1 (a 128-wide
slice of the model dim). Each core computes Q/K/V projections for its head
slice over all tokens, causal attention for its 2 heads, and a partial output
projection (contraction over its 128 x-dims). The host sums the 8 partial
outputs and adds b_o.

All matmuls run in bf16 (full PE rate) with fp32 PSUM accumulation; softmax
runs without max-subtraction (scores are O(10), exp stays in range).

On-device layouts (T = transposed, tokens on the free axis):
  QT/KT: [128 head-dims, 8192 tokens] bf16 in SBUF
  VA:    [128 token-chunk, 64 chunks, 256] bf16; per-head cols = 64 ones
         followed by 64 V dims (the ones give replicated softmax row-sums
         for free, landing in PSUM partitions 0:64)
  Scores are computed transposed, S.T = [k-tokens, q-tokens], so softmax
  normalization lands on the free axis after the attn@V matmul.

Scheduling: both heads' score tiles share one 2-bank PSUM tile so a single
ACTIVATE does exp for both; the two heads' K=64 score matmuls row-tile into
the PE concurrently. Projection (next batch) and output-projection (lagged)
matmuls interleave between attention chunks through two filler queues:
  dq — DMA-dependent projection units, only pumped >= 2 phases after their
       input DMA was issued (a DMA-waiting matmul stalls the in-order PE
       queue, so unready units must never be pumped);
  rq — always-ready output-projection units (XT is resident) + memsets,
       preferred at batch transitions where dq's data is still in flight.
This keeps PE duty high everywhere so the HAM activity monitor never
re-throttles the PE clock to 1.2 GHz mid-kernel.

Output staging: all 8 output-dim chunks of a q-tile collect into one SBUF
tile and ship with a single DMA (16 issues total instead of 128), keeping
the Sync queue free for input streaming.
"""

import sys
import types
from collections import deque

sys.path.insert(0, "/opt/trn_rl_repo")

import numpy as np

# Optional: make run_bass_kernel_spmd(trace=True) work on images whose antenv
# lacks axon_hooks. Harmless if unavailable; kernel() defaults to trace=False.
try:  # pragma: no cover
    import antenv
    if "antenv.axon_hooks" not in sys.modules:
        from trn_agent_boot.trn_boot import _ntff_profile_via_ctypes

        _hook = _ntff_profile_via_ctypes("/opt/axon/libaxon_pjrt.so")
        _mod = types.ModuleType("antenv.axon_hooks")
        _mod.get_axon_ntff_profile_hook = lambda: _hook
        _mod.set_axon_ntff_profile_hook = lambda h: None
        sys.modules["antenv.axon_hooks"] = _mod
        antenv.axon_hooks = _mod
except Exception:
    pass

import concourse.bass as bass
import concourse.bacc as bacc
import concourse.tile as tile
import concourse.mybir as mybir
from concourse.bass_utils import run_bass_kernel_spmd

B, S, D, H = 4, 2048, 1024, 16
DK = D // H          # 64
P = 128
SQ = B * S           # 8192 tokens
NT = SQ // 512       # 16 token tiles of 512
KO = D // P          # 8 contraction chunks
NCORES = 8
F16 = mybir.dt.float16
F32 = mybir.dt.float32

TRACE = False        # set by test.py to capture an NTFF profile
LAST_RESULT = None   # BassKernelResults of the most recent run

MM_DT = mybir.dt.bfloat16
ACT_OPROJ_CAST = False  # all oproj casts on DVE: the ACT queue is the
                        # attention-phase rate cap (exp), keep it pure

_NC = None


def _np_mm_dt():
    if MM_DT == mybir.dt.float16:
        return np.float16
    import ml_dtypes
    return ml_dtypes.bfloat16


def _build():
    nc = bacc.Bacc("TRN2", target_bir_lowering=False, debug=False,
                   num_devices=NCORES)

    qT_d = nc.dram_tensor("qT", [NT, P, KO, 512], MM_DT, kind="ExternalInput")
    kT_d = nc.dram_tensor("kT", [NT, P, KO, 512], MM_DT, kind="ExternalInput")
    vT_d = nc.dram_tensor("vT", [NT, P, KO, 512], MM_DT, kind="ExternalInput")
    wq_d = nc.dram_tensor("wq", [P, KO, P], MM_DT, kind="ExternalInput")
    wk_d = nc.dram_tensor("wk", [P, KO, P], MM_DT, kind="ExternalInput")
    wv_d = nc.dram_tensor("wv", [P, KO, P], MM_DT, kind="ExternalInput")
    wo_d = nc.dram_tensor("wo", [P, KO, P], MM_DT, kind="ExternalInput")
    mk_d = nc.dram_tensor("masks", [P, P], MM_DT, kind="ExternalInput")
    # [tt, p, mo, 512]: matches the SBUF staging tile order so one DMA per
    # q-tile ships all 8 output-dim chunks.
    out_d = nc.dram_tensor("out", [NT, P, KO, 512], F16, kind="ExternalOutput")

    with tile.TileContext(nc) as tc:
        with (
            tc.tile_pool(name="const", bufs=1) as const,
            tc.tile_pool(name="persist", bufs=1) as persist,
            tc.tile_pool(name="stream", bufs=3) as stream,
            tc.tile_pool(name="epool", bufs=6) as epool,
            tc.tile_pool(name="rpool", bufs=2) as rpool,
            tc.tile_pool(name="ostage", bufs=2) as ostage,
            tc.tile_pool(name="vstage", bufs=2) as vstage,
            tc.tile_pool(name="pp", bufs=2, space="PSUM") as pp,
            tc.tile_pool(name="scp", bufs=2, space="PSUM") as scp,
            tc.tile_pool(name="opp", bufs=1, space="PSUM") as opp,
        ):
            wq_t = const.tile([P, KO, P], MM_DT, tag="wq")
            wk_t = const.tile([P, KO, P], MM_DT, tag="wk")
            wv_t = const.tile([P, KO, P], MM_DT, tag="wv")
            wo_t = const.tile([P, KO, P], MM_DT, tag="wo")
            mk_t = const.tile([P, P], MM_DT, tag="mk")

            # Initial loads spread over three engine queues so descriptor
            # issue (~0.6us each) doesn't serialize ahead of the first
            # projection matmul: q/k halves on Sync, v on Vector, weights on
            # Scalar. First-needed first per queue.
            tile0 = {}
            for nm, src in (("qin", qT_d), ("kin", kT_d), ("vin", vT_d)):
                tile0[nm] = stream.tile([P, KO, 512], MM_DT, tag=nm, name=nm)
            # qin in quarters so the first projection matmuls (ko 0:2) can
            # start as soon as ~256KB has landed
            for qq in range(4):
                nc.sync.dma_start(tile0["qin"][:, 2 * qq:2 * qq + 2, :],
                                  qT_d.ap()[0, :, 2 * qq:2 * qq + 2, :])
            for nm, src in (("kin", kT_d), ("vin", vT_d)):
                t0 = tile0[nm]
                for qq in range(4):
                    nc.gpsimd.dma_start(t0[:, 2 * qq:2 * qq + 2, :],
                                        src.ap()[0, :, 2 * qq:2 * qq + 2, :])
            for w_t, w_src in ((wq_t, wq_d), (wk_t, wk_d), (wv_t, wv_d),
                               (wo_t, wo_d)):
                nc.scalar.dma_start(w_t[:], w_src.ap())
            nc.scalar.dma_start(mk_t[:], mk_d.ap())

            ident = const.tile([P, P], MM_DT, tag="ident")
            from concourse.masks import make_identity
            make_identity(nc, ident[:])

            QT = persist.tile([P, SQ], MM_DT, tag="QT")
            KT = persist.tile([P, SQ], MM_DT, tag="KT")
            VA = persist.tile([P, SQ // P, 256], MM_DT, tag="VA")
            XT = persist.tile([P, SQ], MM_DT, tag="XT")

            # ones columns for the row-sum trick; per-chunk layout is
            # [ones 0:64 | A dims 64:128 | ones 128:192 | B dims 192:256]
            # (ones first so the sums land in PSUM partitions 0:64, which
            # reciprocal_approx_fast can read directly — the custom DVE op
            # mis-reads PSUM APs with a partition offset).
            # Only batch-0 chunks are needed before the first attention; the
            # rest is deferred into the ready queue.
            nc.vector.memset(VA[:, 0:16, 0:DK], 1.0)
            nc.vector.memset(VA[:, 0:16, P:P + DK], 1.0)

            # ---- filler queues ---------------------------------------
            # dq: projection units, gated on DMA safety (cur phase must be
            #     >= unit's enqueue phase + 2 so the input tile has landed).
            # rq: oproj units + memsets — data-resident, always safe.
            # need_p(tt) force-drains dq through tile tt (gate bypassed:
            # attention cannot start without its projections anyway).
            dq = deque()   # items: (tt, g_enq, thunk)
            rq = deque()   # items: (key, thunk)
            p_pending = {}
            ms_pending = [0]
            cur_g = [0]

            def dq_push(tt, g_enq, thunk):
                dq.append((tt, g_enq, thunk))
                p_pending[tt] = p_pending.get(tt, 0) + 1

            def dq_pop():
                tt, _, thunk = dq.popleft()
                p_pending[tt] -= 1
                thunk()

            def dq_safe():
                return bool(dq) and cur_g[0] >= dq[0][1] + 1

            def rq_pop():
                key, thunk = rq.popleft()
                if key == "ms":
                    ms_pending[0] -= 1
                thunk()

            def pump(n=1, prefer="d"):
                for _ in range(n):
                    if dq_safe():
                        dq_pop()
                    elif rq:
                        rq_pop()

            def need_p(tt):
                while p_pending.get(tt, 0) > 0:
                    if dq:
                        dq_pop()
                    elif rq:  # pragma: no cover - defensive
                        rq_pop()

            def need_ms():
                while ms_pending[0] > 0:
                    rq_pop()

            def pump_all():
                while dq or rq:
                    if dq:
                        dq_pop()
                    else:
                        rq_pop()

            # ---- projection ------------------------------------------
            def enqueue_proj(tt, g_enq):
                """Queue Q/K/V projections for token tile tt (512 tokens).

                DMAs are issued immediately (at enqueue time) so the data is
                resident by the time the matmul units are pumped.
                """
                if tt == 0:
                    qin, kin, vin = tile0["qin"], tile0["kin"], tile0["vin"]
                else:
                    qin = stream.tile([P, KO, 512], MM_DT, tag="qin", name="qin")
                    kin = stream.tile([P, KO, 512], MM_DT, tag="kin", name="kin")
                    vin = stream.tile([P, KO, 512], MM_DT, tag="vin", name="vin")
                    nc.sync.dma_start(qin[:], qT_d.ap()[tt])
                    nc.sync.dma_start(kin[:], kT_d.ap()[tt])
                    nc.sync.dma_start(vin[:], vT_d.ap()[tt])
                cols = bass.ts(tt, 512)
                state = {}

                def mk_proj(xin, w_t, dst_cols):
                    def half0():
                        ps = pp.tile([P, 512], F32, tag="pp", name="ps")
                        state["ps"] = ps
                        for ko in range(KO // 2):
                            nc.tensor.matmul(ps[:], w_t[:, ko, :],
                                             xin[:, ko, :],
                                             start=(ko == 0), stop=False)

                    def half1():
                        ps = state["ps"]
                        for ko in range(KO // 2, KO):
                            nc.tensor.matmul(ps[:], w_t[:, ko, :],
                                             xin[:, ko, :],
                                             start=False, stop=(ko == KO - 1))
                        nc.vector.tensor_copy(dst_cols, ps[:])
                    return half0, half1

                q0, q1 = mk_proj(qin, wq_t, QT[:, cols])
                k0, k1 = mk_proj(kin, wk_t, KT[:, cols])
                vts = vstage.tile([P, 512], MM_DT, tag="vts", name="vts")
                v0, v1 = mk_proj(vin, wv_t, vts[:])

                # one transpose+copy per unit: a monolithic 4x unit parks
                # DVE copies at the queue head waiting on PE transposes that
                # sit behind queued attention matmuls (head-of-line blocking
                # that stalls unrelated DVE work)
                def mk_vtrans(sub):
                    def vtrans():
                        tp = pp.tile([P, P], MM_DT, tag="pp", name="tp")
                        nc.tensor.transpose(tp[:], vts[:, bass.ts(sub, P)],
                                            ident[:])
                        kc = tt * 4 + sub
                        nc.vector.tensor_copy(
                            VA[:, kc].rearrange("p (a x) -> p a x", a=2)[:, :, DK:P],
                            tp[:].rearrange("p (a x) -> p a x", a=2))
                    return vtrans

                for u in (q0, q1, k0, k1, v0, v1,
                          mk_vtrans(0), mk_vtrans(1), mk_vtrans(2),
                          mk_vtrans(3)):
                    dq_push(tt, g_enq, u)

            # ---- output projection -----------------------------------
            def enqueue_oproj(tt):
                """Queue the output projection of q-tile tt: all 8
                output-dim chunks, contraction over this core's 128 x-dims,
                staged into one SBUF tile and shipped with one DMA."""
                ost = ostage.tile([P, KO, 512], F16, tag="ost", name="ost")

                def mk_unit(mo0):
                    def unit():
                        for mo in (mo0, mo0 + 1):
                            pso = pp.tile([P, 512], F32, tag="pp", name="pso")
                            nc.tensor.matmul(pso[:], wo_t[:, mo, :],
                                             XT[:, bass.ts(tt, 512)],
                                             start=True, stop=True)
                            nc.vector.tensor_copy(ost[:, mo, :], pso[:])
                            if mo == 3 and tt == NT - 1:
                                # final tile: ship the first half early on a
                                # second queue so the tail drain overlaps
                                nc.scalar.dma_start(
                                    out_d.ap()[tt][:, 0:KO // 2, :],
                                    ost[:, 0:KO // 2, :])
                            elif mo == KO - 1:
                                if tt == NT - 1:
                                    nc.sync.dma_start(
                                        out_d.ap()[tt][:, KO // 2:, :],
                                        ost[:, KO // 2:, :])
                                else:
                                    nc.sync.dma_start(out_d.ap()[tt], ost[:])
                    return unit

                for mo0 in (0, 2, 4, 6):
                    rq.append((("o", tt), mk_unit(mo0)))

            # ---- attention -------------------------------------------
            def attention(b, qt):
                """One 512-query tile of causal attention, both heads.

                Software-pipelined three chunks deep: the attn@V matmuls of
                chunk kc are emitted after the scores matmuls of chunk kc+3,
                so the ScalarE exp (+ GpSimd mask) of chunk kc has three full
                chunk periods of latency slack, and filler units are pumped
                per chunk to keep the PE queue dense.
                """
                qcols = bass.ds(b * S + qt * 512, 512)
                nkc = 4 * qt + 4
                ops = opp.tile([P, 2, 512], F32, tag="op", name="ops")
                pipe = deque()
                # dq first whenever its data has landed: prompt projection
                # consumption recycles stream-pool slots, which is what keeps
                # the input DMA pipeline moving. rq fills genuine droughts.
                prefer = "d"

                def scores_exp(kc):
                    kcols = bass.ds(b * S + kc * P, P)
                    j = kc - 4 * qt
                    co = max(j, 0) * P  # valid q-columns start here (causal)
                    w = 512 - co
                    ssc = scp.tile([P, 2, 512], F32, tag="sc", name="ssc")
                    for h in range(2):
                        rb = h * DK
                        nc.tensor.matmul(
                            ssc[:, h, co:],
                            KT[rb:rb + DK, kcols],
                            QT[rb:rb + DK, bass.ds(b * S + qt * 512 + co, w)],
                            start=True, stop=True)
                    e_t = epool.tile([P, 2, 512], MM_DT, tag="e", name="e_t")
                    nc.scalar.activation(e_t[:, :, co:], ssc[:, :, co:],
                                         mybir.ActivationFunctionType.Exp,
                                         scale=0.125)
                    if j >= 0:
                        # one op for both heads: mask broadcast over the
                        # head axis (halves the GpSimd latency in the
                        # exp -> mask -> attn chain on diagonal chunks)
                        nc.gpsimd.tensor_mul(
                            e_t[:, :, co:co + P], e_t[:, :, co:co + P],
                            mk_t[:].unsqueeze(1).broadcast_to((P, 2, P)))
                    pipe.append((e_t, co))

                def attn_mm(kc):
                    e_t, co = pipe.popleft()
                    gkc = b * (S // P) + kc
                    for h in range(2):
                        nc.tensor.matmul(ops[:, h, co:],
                                         VA[:, gkc, bass.ts(h, P)],
                                         e_t[:, h, co:],
                                         start=(kc == 0), stop=(kc == nkc - 1))

                rate = 2 if nkc <= 8 else 1
                # final phase pumps thinner so ~2 ready units survive into
                # pump_all and bridge the last tile's normalize latency
                # (otherwise the PE idles there, HAM re-throttles, and the
                # final output projection runs at half clock)
                thin = (b == B - 1 and qt == 3)
                scores_exp(0)
                scores_exp(1)
                pump(1, prefer)  # filler before sc(2)'s PSUM-slot reuse
                scores_exp(2)
                pump(rate, prefer)
                for kc in range(3, nkc):
                    scores_exp(kc)
                    attn_mm(kc - 3)
                    if not (thin and kc % 2 == 0):
                        pump(rate, prefer)
                attn_mm(nkc - 3)
                pump(rate, prefer)
                attn_mm(nkc - 2)
                pump(1, prefer)
                attn_mm(nkc - 1)
                pump(2, prefer)  # keep the PE fed while the epilogue frees ops

                # normalize: sums are replicated in rows 0:64 of each head
                # (ones-first VA layout), attn values in rows 64:128
                r_t = rpool.tile([DK, 2, 512], F32, tag="r", name="r_t")
                nc.vector.reciprocal_approx_fast(r_t[:], ops[0:DK, :, :])
                for h in range(2):
                    nc.vector.tensor_mul(XT[h * DK:(h + 1) * DK, qcols],
                                         ops[DK:P, h, :], r_t[:, h, :])

            # ---- schedule --------------------------------------------
            for tt in range(4):
                enqueue_proj(tt, tt - 4)
            rq.append(("ms", lambda: nc.vector.memset(VA[:, 16:64, 0:DK], 1.0)))
            rq.append(("ms",
                       lambda: nc.vector.memset(VA[:, 16:64, P:P + DK], 1.0)))
            ms_pending[0] = 2
            for b in range(B):
                for qt in range(4):
                    g = 4 * b + qt
                    cur_g[0] = g
                    if b + 1 < B:
                        enqueue_proj(4 * (b + 1) + qt, g)
                    # oproj runs 4 tiles behind (XT persists), with catch-up
                    # at g=14/15: batch 3 has no next-batch projections to
                    # pump, so the backlog is what keeps its attention
                    # phases dense enough that HAM never re-throttles
                    if 4 <= g <= 13:
                        enqueue_oproj(g - 4)
                    elif g == 14:
                        enqueue_oproj(10)
                        enqueue_oproj(11)
                    elif g == 15:
                        enqueue_oproj(12)
                        enqueue_oproj(13)
                        enqueue_oproj(14)
                    for t in range(qt + 1):
                        need_p(4 * b + t)
                    if b > 0:
                        need_ms()
                    attention(b, qt)
            enqueue_oproj(NT - 1)
            pump_all()

    nc.compile()
    return nc


def _get_nc():
    global _NC
    if _NC is None:
        _NC = _build()
    return _NC


def _to_tiled_T(x2):
    """[SQ, D] fp32 -> [NT, 128, KO, 512] bf16 with x[g, d] at
    [g//512, d%128, d//128, g%512]."""
    xh = x2.astype(_np_mm_dt())
    return np.ascontiguousarray(
        xh.reshape(NT, 512, KO, P).transpose(0, 3, 2, 1))


def _weight_T(w_slice):
    """[128 out, 1024 in] -> [128 p, KO, 128 m] bf16 with W[m, d] at
    [d%128, d//128, m]."""
    return np.ascontiguousarray(
        w_slice.T.reshape(KO, P, P).transpose(1, 0, 2)).astype(_np_mm_dt())


def kernel(q, k, v, mask, W_q, W_k, W_v, W_o, b_o):
    global LAST_RESULT
    nc = _get_nc()

    qT = _to_tiled_T(np.asarray(q, np.float32).reshape(SQ, D))
    kT = _to_tiled_T(np.asarray(k, np.float32).reshape(SQ, D))
    vT = _to_tiled_T(np.asarray(v, np.float32).reshape(SQ, D))

    p_idx = np.arange(P)[:, None]
    f_idx = np.arange(P)[None, :]
    masks = (f_idx >= p_idx).astype(_np_mm_dt())

    W_q = np.asarray(W_q, np.float32)
    W_k = np.asarray(W_k, np.float32)
    W_v = np.asarray(W_v, np.float32)
    W_o = np.asarray(W_o, np.float32)

    in_maps = []
    for c in range(NCORES):
        cs = slice(c * P, (c + 1) * P)
        in_maps.append({
            "qT": qT, "kT": kT, "vT": vT, "masks": masks,
            "wq": _weight_T(W_q[cs, :]),
            "wk": _weight_T(W_k[cs, :]),
            "wv": _weight_T(W_v[cs, :]),
            # [k, mo, m] = W_o[mo*128+m, c*128+k]
            "wo": np.ascontiguousarray(
                W_o[:, cs].reshape(KO, P, P).transpose(2, 0, 1)
            ).astype(_np_mm_dt()),
        })

    res = run_bass_kernel_spmd(nc, in_maps, core_ids=list(range(NCORES)),
                               trace=TRACE)
    LAST_RESULT = res

    acc = np.zeros((SQ, D), np.float32)
    for c in range(NCORES):
        # out: [NT, P, KO, 512] -> [KO*P, NT*512] = [D, SQ]
        partial_T = res.results[c]["out"].transpose(2, 1, 0, 3).reshape(D, SQ)
        acc += partial_T.T.astype(np.float32)
    acc += np.asarray(b_o, np.float32)
    return acc.reshape(B, S, D)


# revision 36
# speedup vs baseline: 1.0213x; 1.0213x over previous
"""Multi-head attention (B=4, S=2048, D=1024, H=16) on 8 Trainium2 NeuronCores.

Sharding: tensor-parallel over heads. Core c owns heads 2c, 2c+1 (a 128-wide
slice of the model dim). Each core computes Q/K/V projections for its head
slice over all tokens, causal attention for its 2 heads, and a partial output
projection (contraction over its 128 x-dims). The host sums the 8 partial
outputs and adds b_o.

All matmuls run in bf16 (full PE rate) with fp32 PSUM accumulation; softmax
runs without max-subtraction (scores are O(10), exp stays in range).

On-device layouts (T = transposed, tokens on the free axis):
  QT/KT: [128 head-dims, 8192 tokens] bf16 in SBUF
  VA:    [128 token-chunk, 64 chunks, 256] bf16; per-head cols = 64 ones
         followed by 64 V dims (the ones give replicated softmax row-sums
         for free, landing in PSUM partitions 0:64)
  Scores are computed transposed, S.T = [k-tokens, q-tokens], so softmax
  normalization lands on the free axis after the attn@V matmul.

Scheduling: both heads' score tiles share one 2-bank PSUM tile so a single
ACTIVATE does exp for both; the two heads' K=64 score matmuls row-tile into
the PE concurrently. Projection (next batch) and output-projection (lagged)
matmuls interleave between attention chunks through two filler queues:
  dq — DMA-dependent projection units, only pumped >= 2 phases after their
       input DMA was issued (a DMA-waiting matmul stalls the in-order PE
       queue, so unready units must never be pumped);
  rq — always-ready output-projection units (XT is resident) + memsets,
       preferred at batch transitions where dq's data is still in flight.
This keeps PE duty high everywhere so the HAM activity monitor never
re-throttles the PE clock to 1.2 GHz mid-kernel.

Output staging: all 8 output-dim chunks of a q-tile collect into one SBUF
tile and ship with a single DMA (16 issues total instead of 128), keeping
the Sync queue free for input streaming.
"""

import sys
import types
from collections import deque

sys.path.insert(0, "/opt/trn_rl_repo")

import numpy as np

# Optional: make run_bass_kernel_spmd(trace=True) work on images whose antenv
# lacks axon_hooks. Harmless if unavailable; kernel() defaults to trace=False.
try:  # pragma: no cover
    import antenv
    if "antenv.axon_hooks" not in sys.modules:
        from trn_agent_boot.trn_boot import _ntff_profile_via_ctypes

        _hook = _ntff_profile_via_ctypes("/opt/axon/libaxon_pjrt.so")
        _mod = types.ModuleType("antenv.axon_hooks")
        _mod.get_axon_ntff_profile_hook = lambda: _hook
        _mod.set_axon_ntff_profile_hook = lambda h: None
        sys.modules["antenv.axon_hooks"] = _mod
        antenv.axon_hooks = _mod
except Exception:
    pass

import concourse.bass as bass
import concourse.bacc as bacc
import concourse.tile as tile
import concourse.mybir as mybir
from concourse.bass_utils import run_bass_kernel_spmd

B, S, D, H = 4, 2048, 1024, 16
DK = D // H          # 64
P = 128
SQ = B * S           # 8192 tokens
NT = SQ // 512       # 16 token tiles of 512
KO = D // P          # 8 contraction chunks
NCORES = 8
F16 = mybir.dt.float16
F32 = mybir.dt.float32

TRACE = False        # set by test.py to capture an NTFF profile
LAST_RESULT = None   # BassKernelResults of the most recent run

MM_DT = mybir.dt.bfloat16
ACT_OPROJ_CAST = False  # all oproj casts on DVE: the ACT queue is the
                        # attention-phase rate cap (exp), keep it pure

_NC = None


def _np_mm_dt():
    if MM_DT == mybir.dt.float16:
        return np.float16
    import ml_dtypes
    return ml_dtypes.bfloat16


def _build():
    nc = bacc.Bacc("TRN2", target_bir_lowering=False, debug=False,
                   num_devices=NCORES)

    qT_d = nc.dram_tensor("qT", [NT, P, KO, 512], MM_DT, kind="ExternalInput")
    kT_d = nc.dram_tensor("kT", [NT, P, KO, 512], MM_DT, kind="ExternalInput")
    vT_d = nc.dram_tensor("vT", [NT, P, KO, 512], MM_DT, kind="ExternalInput")
    # all four weight matrices packed into one tensor: DMA engines are
    # packet-rate-bound (~50ns per per-partition segment), so one 8KB/
    # partition descriptor moves 4x the bytes of four 2KB/partition ones
    w_d = nc.dram_tensor("w", [P, 4, KO, P], MM_DT, kind="ExternalInput")
    mk_d = nc.dram_tensor("masks", [P, P], MM_DT, kind="ExternalInput")
    # [tt, p, mo, 512]: matches the SBUF staging tile order so one DMA per
    # q-tile ships all 8 output-dim chunks.
    out_d = nc.dram_tensor("out", [NT, P, KO, 512], F16, kind="ExternalOutput")

    with tile.TileContext(nc) as tc:
        with (
            tc.tile_pool(name="const", bufs=1) as const,
            tc.tile_pool(name="persist", bufs=1) as persist,
            tc.tile_pool(name="stream", bufs=3) as stream,
            tc.tile_pool(name="epool", bufs=6) as epool,
            tc.tile_pool(name="rpool", bufs=2) as rpool,
            tc.tile_pool(name="ostage", bufs=2) as ostage,
            tc.tile_pool(name="vstage", bufs=2) as vstage,
            tc.tile_pool(name="pp", bufs=2, space="PSUM") as pp,
            tc.tile_pool(name="scp", bufs=2, space="PSUM") as scp,
            tc.tile_pool(name="opp", bufs=1, space="PSUM") as opp,
        ):
            w_t = const.tile([P, 4, KO, P], MM_DT, tag="w")
            wq_t, wk_t, wv_t, wo_t = (w_t[:, j] for j in range(4))
            mk_t = const.tile([P, P], MM_DT, tag="mk")

            # Initial loads: few LARGE descriptors spread over the three
            # DMA-capable queues. Per-queue service is packet-rate-bound, so
            # splitting a tile into small segments serializes ~5us apiece on
            # one queue (measured) — full-tile 8KB/partition descriptors are
            # ~3x faster per byte. q on Sync, k on GpSimd, v split across
            # both behind them, weights (one descriptor) + mask on Scalar.
            tile0 = {}
            for nm, src in (("qin", qT_d), ("kin", kT_d), ("vin", vT_d)):
                tile0[nm] = stream.tile([P, KO, 512], MM_DT, tag=nm, name=nm)
            nc.sync.dma_start(tile0["qin"][:], qT_d.ap()[0])
            nc.gpsimd.dma_start(tile0["kin"][:], kT_d.ap()[0])
            nc.sync.dma_start(tile0["vin"][:, 0:KO // 2, :],
                              vT_d.ap()[0, :, 0:KO // 2, :])
            nc.gpsimd.dma_start(tile0["vin"][:, KO // 2:, :],
                                vT_d.ap()[0, :, KO // 2:, :])
            nc.scalar.dma_start(w_t[:], w_d.ap())
            nc.scalar.dma_start(mk_t[:], mk_d.ap())

            ident = const.tile([P, P], MM_DT, tag="ident")
            from concourse.masks import make_identity
            make_identity(nc, ident[:])

            QT = persist.tile([P, SQ], MM_DT, tag="QT")
            KT = persist.tile([P, SQ], MM_DT, tag="KT")
            VA = persist.tile([P, SQ // P, 256], MM_DT, tag="VA")
            XT = persist.tile([P, SQ], MM_DT, tag="XT")

            # ones columns for the row-sum trick; per-chunk layout is
            # [ones 0:64 | A dims 64:128 | ones 128:192 | B dims 192:256]
            # (ones first so the sums land in PSUM partitions 0:64, which
            # reciprocal_approx_fast can read directly — the custom DVE op
            # mis-reads PSUM APs with a partition offset).
            # Only batch-0 chunks are needed before the first attention; the
            # rest is deferred into the ready queue.
            nc.vector.memset(VA[:, 0:16, 0:DK], 1.0)
            nc.vector.memset(VA[:, 0:16, P:P + DK], 1.0)

            # ---- filler queues ---------------------------------------
            # dq: projection units, gated on DMA safety (cur phase must be
            #     >= unit's enqueue phase + 2 so the input tile has landed).
            # rq: oproj units + memsets — data-resident, always safe.
            # need_p(tt) force-drains dq through tile tt (gate bypassed:
            # attention cannot start without its projections anyway).
            dq = deque()   # items: (tt, g_enq, thunk)
            rq = deque()   # items: (key, thunk)
            p_pending = {}
            ms_pending = [0]
            cur_g = [0]

            def dq_push(tt, g_enq, thunk):
                dq.append((tt, g_enq, thunk))
                p_pending[tt] = p_pending.get(tt, 0) + 1

            def dq_pop():
                tt, _, thunk = dq.popleft()
                p_pending[tt] -= 1
                thunk()

            def dq_safe():
                return bool(dq) and cur_g[0] >= dq[0][1] + 1

            def rq_pop():
                key, thunk = rq.popleft()
                if key == "ms":
                    ms_pending[0] -= 1
                thunk()

            def pump(n=1, prefer="d"):
                for _ in range(n):
                    if dq_safe():
                        dq_pop()
                    elif rq:
                        rq_pop()

            def need_p(tt):
                while p_pending.get(tt, 0) > 0:
                    if dq:
                        dq_pop()
                    elif rq:  # pragma: no cover - defensive
                        rq_pop()

            def need_ms():
                while ms_pending[0] > 0:
                    rq_pop()

            def pump_all():
                while dq or rq:
                    if dq:
                        dq_pop()
                    else:
                        rq_pop()

            # ---- projection ------------------------------------------
            def enqueue_proj(tt, g_enq):
                """Queue Q/K/V projections for token tile tt (512 tokens).

                DMAs are issued immediately (at enqueue time) so the data is
                resident by the time the matmul units are pumped.
                """
                if tt == 0:
                    qin, kin, vin = tile0["qin"], tile0["kin"], tile0["vin"]
                else:
                    qin = stream.tile([P, KO, 512], MM_DT, tag="qin", name="qin")
                    kin = stream.tile([P, KO, 512], MM_DT, tag="kin", name="kin")
                    vin = stream.tile([P, KO, 512], MM_DT, tag="vin", name="vin")
                    nc.sync.dma_start(qin[:], qT_d.ap()[tt])
                    nc.sync.dma_start(kin[:], kT_d.ap()[tt])
                    nc.sync.dma_start(vin[:], vT_d.ap()[tt])
                cols = bass.ts(tt, 512)
                state = {}

                def mk_proj(xin, w_t, dst_cols):
                    def half0():
                        ps = pp.tile([P, 512], F32, tag="pp", name="ps")
                        state["ps"] = ps
                        for ko in range(KO // 2):
                            nc.tensor.matmul(ps[:], w_t[:, ko, :],
                                             xin[:, ko, :],
                                             start=(ko == 0), stop=False)

                    def half1():
                        ps = state["ps"]
                        for ko in range(KO // 2, KO):
                            nc.tensor.matmul(ps[:], w_t[:, ko, :],
                                             xin[:, ko, :],
                                             start=False, stop=(ko == KO - 1))
                        nc.vector.tensor_copy(dst_cols, ps[:])
                    return half0, half1

                q0, q1 = mk_proj(qin, wq_t, QT[:, cols])
                k0, k1 = mk_proj(kin, wk_t, KT[:, cols])
                vts = vstage.tile([P, 512], MM_DT, tag="vts", name="vts")
                v0, v1 = mk_proj(vin, wv_t, vts[:])

                # one transpose+copy per unit: a monolithic 4x unit parks
                # DVE copies at the queue head waiting on PE transposes that
                # sit behind queued attention matmuls (head-of-line blocking
                # that stalls unrelated DVE work)
                def mk_vtrans(sub):
                    def vtrans():
                        tp = pp.tile([P, P], MM_DT, tag="pp", name="tp")
                        nc.tensor.transpose(tp[:], vts[:, bass.ts(sub, P)],
                                            ident[:])
                        kc = tt * 4 + sub
                        nc.vector.tensor_copy(
                            VA[:, kc].rearrange("p (a x) -> p a x", a=2)[:, :, DK:P],
                            tp[:].rearrange("p (a x) -> p a x", a=2))
                    return vtrans

                for u in (q0, q1, k0, k1, v0, v1,
                          mk_vtrans(0), mk_vtrans(1), mk_vtrans(2),
                          mk_vtrans(3)):
                    dq_push(tt, g_enq, u)

            # ---- output projection -----------------------------------
            def enqueue_oproj(tt):
                """Queue the output projection of q-tile tt: all 8
                output-dim chunks, contraction over this core's 128 x-dims,
                staged into one SBUF tile and shipped with one DMA."""
                ost = ostage.tile([P, KO, 512], F16, tag="ost", name="ost")

                def mk_unit(mo0):
                    def unit():
                        for mo in (mo0, mo0 + 1):
                            pso = pp.tile([P, 512], F32, tag="pp", name="pso")
                            nc.tensor.matmul(pso[:], wo_t[:, mo, :],
                                             XT[:, bass.ts(tt, 512)],
                                             start=True, stop=True)
                            if tt == NT - 1 and mo % 2 == 1:
                                # final tile: ACT is idle in the tail and the
                                # pure-DVE cast chain (8x690ns serial) was
                                # the tail's critical path — alternate, and
                                # ship quarters as they complete
                                nc.scalar.copy(ost[:, mo, :], pso[:])
                                q_dst = out_d.ap()[tt][:, mo - 1:mo + 1, :]
                                q_src = ost[:, mo - 1:mo + 1, :]
                                if mo % 4 == 1:
                                    nc.scalar.dma_start(q_dst, q_src)
                                else:
                                    nc.sync.dma_start(q_dst, q_src)
                            else:
                                nc.vector.tensor_copy(ost[:, mo, :], pso[:])
                                if mo == KO - 1:
                                    nc.sync.dma_start(out_d.ap()[tt], ost[:])
                    return unit

                for mo0 in (0, 2, 4, 6):
                    rq.append((("o", tt), mk_unit(mo0)))

            # ---- attention -------------------------------------------
            def attention(b, qt):
                """One 512-query tile of causal attention, both heads.

                Software-pipelined three chunks deep: the attn@V matmuls of
                chunk kc are emitted after the scores matmuls of chunk kc+3,
                so the ScalarE exp (+ GpSimd mask) of chunk kc has three full
                chunk periods of latency slack, and filler units are pumped
                per chunk to keep the PE queue dense.
                """
                qcols = bass.ds(b * S + qt * 512, 512)
                nkc = 4 * qt + 4
                ops = opp.tile([P, 2, 512], F32, tag="op", name="ops")
                pipe = deque()
                # dq first whenever its data has landed: prompt projection
                # consumption recycles stream-pool slots, which is what keeps
                # the input DMA pipeline moving. rq fills genuine droughts.
                prefer = "d"

                def scores_exp(kc):
                    kcols = bass.ds(b * S + kc * P, P)
                    j = kc - 4 * qt
                    co = max(j, 0) * P  # valid q-columns start here (causal)
                    w = 512 - co
                    ssc = scp.tile([P, 2, 512], F32, tag="sc", name="ssc")
                    for h in range(2):
                        rb = h * DK
                        nc.tensor.matmul(
                            ssc[:, h, co:],
                            KT[rb:rb + DK, kcols],
                            QT[rb:rb + DK, bass.ds(b * S + qt * 512 + co, w)],
                            start=True, stop=True)
                    e_t = epool.tile([P, 2, 512], MM_DT, tag="e", name="e_t")
                    nc.scalar.activation(e_t[:, :, co:], ssc[:, :, co:],
                                         mybir.ActivationFunctionType.Exp,
                                         scale=0.125)
                    if j >= 0:
                        # one op for both heads: mask broadcast over the
                        # head axis (halves the GpSimd latency in the
                        # exp -> mask -> attn chain on diagonal chunks)
                        nc.gpsimd.tensor_mul(
                            e_t[:, :, co:co + P], e_t[:, :, co:co + P],
                            mk_t[:].unsqueeze(1).broadcast_to((P, 2, P)))
                    pipe.append((e_t, co))

                def attn_mm(kc):
                    e_t, co = pipe.popleft()
                    gkc = b * (S // P) + kc
                    for h in range(2):
                        nc.tensor.matmul(ops[:, h, co:],
                                         VA[:, gkc, bass.ts(h, P)],
                                         e_t[:, h, co:],
                                         start=(kc == 0), stop=(kc == nkc - 1))

                rate = 2 if nkc <= 8 else 1
                scores_exp(0)
                scores_exp(1)
                pump(1, prefer)  # filler before sc(2)'s PSUM-slot reuse
                scores_exp(2)
                pump(rate, prefer)
                for kc in range(3, nkc):
                    scores_exp(kc)
                    attn_mm(kc - 3)
                    pump(rate, prefer)
                attn_mm(nkc - 3)
                pump(rate, prefer)
                attn_mm(nkc - 2)
                pump(1, prefer)
                attn_mm(nkc - 1)
                pump(2, prefer)  # keep the PE fed while the epilogue frees ops

                # normalize: sums are replicated in rows 0:64 of each head
                # (ones-first VA layout), attn values in rows 64:128
                r_t = rpool.tile([DK, 2, 512], F32, tag="r", name="r_t")
                nc.vector.reciprocal_approx_fast(r_t[:], ops[0:DK, :, :])
                for h in range(2):
                    nc.vector.tensor_mul(XT[h * DK:(h + 1) * DK, qcols],
                                         ops[DK:P, h, :], r_t[:, h, :])

            # ---- schedule --------------------------------------------
            for tt in range(4):
                enqueue_proj(tt, tt - 4)
            rq.append(("ms", lambda: nc.vector.memset(VA[:, 16:64, 0:DK], 1.0)))
            rq.append(("ms",
                       lambda: nc.vector.memset(VA[:, 16:64, P:P + DK], 1.0)))
            ms_pending[0] = 2
            for b in range(B):
                for qt in range(4):
                    g = 4 * b + qt
                    cur_g[0] = g
                    if b + 1 < B:
                        enqueue_proj(4 * (b + 1) + qt, g)
                    # oproj runs 4 tiles behind (XT persists), with catch-up
                    # at g=14/15: batch 3 has no next-batch projections to
                    # pump, so the backlog is what keeps its attention
                    # phases dense enough that HAM never re-throttles
                    if 4 <= g <= 13:
                        enqueue_oproj(g - 4)
                    elif g == 14:
                        enqueue_oproj(10)
                        enqueue_oproj(11)
                    elif g == 15:
                        enqueue_oproj(12)
                        enqueue_oproj(13)
                        enqueue_oproj(14)
                    for t in range(qt + 1):
                        need_p(4 * b + t)
                    if b > 0:
                        need_ms()
                    attention(b, qt)
            enqueue_oproj(NT - 1)
            pump_all()

    nc.compile()
    return nc


def _get_nc():
    global _NC
    if _NC is None:
        _NC = _build()
    return _NC


def _to_tiled_T(x2):
    """[SQ, D] fp32 -> [NT, 128, KO, 512] bf16 with x[g, d] at
    [g//512, d%128, d//128, g%512]."""
    xh = x2.astype(_np_mm_dt())
    return np.ascontiguousarray(
        xh.reshape(NT, 512, KO, P).transpose(0, 3, 2, 1))


def _weight_T(w_slice):
    """[128 out, 1024 in] -> [128 p, KO, 128 m] bf16 with W[m, d] at
    [d%128, d//128, m]."""
    return np.ascontiguousarray(
        w_slice.T.reshape(KO, P, P).transpose(1, 0, 2)).astype(_np_mm_dt())


def kernel(q, k, v, mask, W_q, W_k, W_v, W_o, b_o):
    global LAST_RESULT
    nc = _get_nc()

    qT = _to_tiled_T(np.asarray(q, np.float32).reshape(SQ, D))
    kT = _to_tiled_T(np.asarray(k, np.float32).reshape(SQ, D))
    vT = _to_tiled_T(np.asarray(v, np.float32).reshape(SQ, D))

    p_idx = np.arange(P)[:, None]
    f_idx = np.arange(P)[None, :]
    masks = (f_idx >= p_idx).astype(_np_mm_dt())

    W_q = np.asarray(W_q, np.float32)
    W_k = np.asarray(W_k, np.float32)
    W_v = np.asarray(W_v, np.float32)
    W_o = np.asarray(W_o, np.float32)

    in_maps = []
    for c in range(NCORES):
        cs = slice(c * P, (c + 1) * P)
        # wo: [k, mo, m] = W_o[mo*128+m, c*128+k]
        wo_c = np.ascontiguousarray(
            W_o[:, cs].reshape(KO, P, P).transpose(2, 0, 1)
        ).astype(_np_mm_dt())
        w_all = np.ascontiguousarray(np.stack(
            [_weight_T(W_q[cs, :]), _weight_T(W_k[cs, :]),
             _weight_T(W_v[cs, :]), wo_c], axis=1))
        in_maps.append({"qT": qT, "kT": kT, "vT": vT, "masks": masks,
                        "w": w_all})

    res = run_bass_kernel_spmd(nc, in_maps, core_ids=list(range(NCORES)),
                               trace=TRACE)
    LAST_RESULT = res

    acc = np.zeros((SQ, D), np.float32)
    for c in range(NCORES):
        # out: [NT, P, KO, 512] -> [KO*P, NT*512] = [D, SQ]
        partial_T = res.results[c]["out"].transpose(2, 1, 0, 3).reshape(D, SQ)
        acc += partial_T.T.astype(np.float32)
    acc += np.asarray(b_o, np.float32)
    return acc.reshape(B, S, D)


# revision 37
# speedup vs baseline: 1.0239x; 1.0025x over previous
"""Multi-head attention (B=4, S=2048, D=1024, H=16) on 8 Trainium2 NeuronCores.

Sharding: tensor-parallel over heads. Core c owns heads 2c, 2c+1 (a 128-wide
slice of the model dim). Each core computes Q/K/V projections for its head
slice over all tokens, causal attention for its 2 heads, and a partial output
projection (contraction over its 128 x-dims). The host sums the 8 partial
outputs and adds b_o.

All matmuls run in bf16 (full PE rate) with fp32 PSUM accumulation; softmax
runs without max-subtraction (scores are O(10), exp stays in range).

On-device layouts (T = transposed, tokens on the free axis):
  QT/KT: [128 head-dims, 8192 tokens] bf16 in SBUF
  VA:    [128 token-chunk, 64 chunks, 256] bf16; per-head cols = 64 ones
         followed by 64 V dims (the ones give replicated softmax row-sums
         for free, landing in PSUM partitions 0:64)
  Scores are computed transposed, S.T = [k-tokens, q-tokens], so softmax
  normalization lands on the free axis after the attn@V matmul.

Scheduling: both heads' score tiles share one 2-bank PSUM tile so a single
ACTIVATE does exp for both; the two heads' K=64 score matmuls row-tile into
the PE concurrently. Projection (next batch) and output-projection (lagged)
matmuls interleave between attention chunks through two filler queues:
  dq — DMA-dependent projection units, only pumped >= 2 phases after their
       input DMA was issued (a DMA-waiting matmul stalls the in-order PE
       queue, so unready units must never be pumped);
  rq — always-ready output-projection units (XT is resident) + memsets,
       preferred at batch transitions where dq's data is still in flight.
This keeps PE duty high everywhere so the HAM activity monitor never
re-throttles the PE clock to 1.2 GHz mid-kernel.

Output staging: all 8 output-dim chunks of a q-tile collect into one SBUF
tile and ship with a single DMA (16 issues total instead of 128), keeping
the Sync queue free for input streaming.
"""

import sys
import types
from collections import deque

sys.path.insert(0, "/opt/trn_rl_repo")

import numpy as np

# Optional: make run_bass_kernel_spmd(trace=True) work on images whose antenv
# lacks axon_hooks. Harmless if unavailable; kernel() defaults to trace=False.
try:  # pragma: no cover
    import antenv
    if "antenv.axon_hooks" not in sys.modules:
        from trn_agent_boot.trn_boot import _ntff_profile_via_ctypes

        _hook = _ntff_profile_via_ctypes("/opt/axon/libaxon_pjrt.so")
        _mod = types.ModuleType("antenv.axon_hooks")
        _mod.get_axon_ntff_profile_hook = lambda: _hook
        _mod.set_axon_ntff_profile_hook = lambda h: None
        sys.modules["antenv.axon_hooks"] = _mod
        antenv.axon_hooks = _mod
except Exception:
    pass

import concourse.bass as bass
import concourse.bacc as bacc
import concourse.tile as tile
import concourse.mybir as mybir
from concourse.bass_utils import run_bass_kernel_spmd

B, S, D, H = 4, 2048, 1024, 16
DK = D // H          # 64
P = 128
SQ = B * S           # 8192 tokens
NT = SQ // 512       # 16 token tiles of 512
KO = D // P          # 8 contraction chunks
NCORES = 8
F16 = mybir.dt.float16
F32 = mybir.dt.float32

TRACE = False        # set by test.py to capture an NTFF profile
LAST_RESULT = None   # BassKernelResults of the most recent run

MM_DT = mybir.dt.bfloat16
ACT_OPROJ_CAST = False  # all oproj casts on DVE: the ACT queue is the
                        # attention-phase rate cap (exp), keep it pure

_NC = None


def _np_mm_dt():
    if MM_DT == mybir.dt.float16:
        return np.float16
    import ml_dtypes
    return ml_dtypes.bfloat16


def _build():
    nc = bacc.Bacc("TRN2", target_bir_lowering=False, debug=False,
                   num_devices=NCORES)

    qT_d = nc.dram_tensor("qT", [NT, P, KO, 512], MM_DT, kind="ExternalInput")
    kT_d = nc.dram_tensor("kT", [NT, P, KO, 512], MM_DT, kind="ExternalInput")
    vT_d = nc.dram_tensor("vT", [NT, P, KO, 512], MM_DT, kind="ExternalInput")
    # all four weight matrices packed into one tensor: DMA engines are
    # packet-rate-bound (~50ns per per-partition segment), so one 8KB/
    # partition descriptor moves 4x the bytes of four 2KB/partition ones
    w_d = nc.dram_tensor("w", [P, 4, KO, P], MM_DT, kind="ExternalInput")
    mk_d = nc.dram_tensor("masks", [P, P], MM_DT, kind="ExternalInput")
    # [tt, p, mo, 512]: matches the SBUF staging tile order so one DMA per
    # q-tile ships all 8 output-dim chunks.
    out_d = nc.dram_tensor("out", [NT, P, KO, 512], F16, kind="ExternalOutput")

    with tile.TileContext(nc) as tc:
        with (
            tc.tile_pool(name="const", bufs=1) as const,
            tc.tile_pool(name="persist", bufs=1) as persist,
            tc.tile_pool(name="stream", bufs=3) as stream,
            tc.tile_pool(name="epool", bufs=6) as epool,
            tc.tile_pool(name="rpool", bufs=2) as rpool,
            tc.tile_pool(name="ostage", bufs=2) as ostage,
            tc.tile_pool(name="vstage", bufs=2) as vstage,
            tc.tile_pool(name="pp", bufs=2, space="PSUM") as pp,
            tc.tile_pool(name="scp", bufs=2, space="PSUM") as scp,
            tc.tile_pool(name="opp", bufs=1, space="PSUM") as opp,
        ):
            w_t = const.tile([P, 4, KO, P], MM_DT, tag="w")
            wq_t, wk_t, wv_t, wo_t = (w_t[:, j] for j in range(4))
            mk_t = const.tile([P, P], MM_DT, tag="mk")

            # Initial loads: few LARGE descriptors spread over the three
            # DMA-capable queues. Per-queue service is packet-rate-bound, so
            # splitting a tile into small segments serializes ~5us apiece on
            # one queue (measured) — full-tile 8KB/partition descriptors are
            # ~3x faster per byte. q on Sync, k on GpSimd, v split across
            # both behind them, weights (one descriptor) + mask on Scalar.
            tile0 = {}
            for nm, src in (("qin", qT_d), ("kin", kT_d), ("vin", vT_d)):
                tile0[nm] = stream.tile([P, KO, 512], MM_DT, tag=nm, name=nm)
            # ~1MB per queue, ordered by first consumer (q, then k, then v):
            # per-queue service rate is the startup bound, so balance bytes
            nc.sync.dma_start(tile0["qin"][:, 0:KO // 2, :],
                              qT_d.ap()[0, :, 0:KO // 2, :])
            nc.gpsimd.dma_start(tile0["qin"][:, KO // 2:, :],
                                qT_d.ap()[0, :, KO // 2:, :])
            nc.gpsimd.dma_start(tile0["kin"][:, 0:KO // 2, :],
                                kT_d.ap()[0, :, 0:KO // 2, :])
            nc.sync.dma_start(tile0["kin"][:, KO // 2:, :],
                              kT_d.ap()[0, :, KO // 2:, :])
            nc.scalar.dma_start(w_t[:], w_d.ap())
            nc.scalar.dma_start(tile0["vin"][:, 0:KO // 2, :],
                                vT_d.ap()[0, :, 0:KO // 2, :])
            nc.gpsimd.dma_start(tile0["vin"][:, KO // 2:KO - 2, :],
                                vT_d.ap()[0, :, KO // 2:KO - 2, :])
            nc.sync.dma_start(tile0["vin"][:, KO - 2:, :],
                              vT_d.ap()[0, :, KO - 2:, :])
            nc.scalar.dma_start(mk_t[:], mk_d.ap())

            ident = const.tile([P, P], MM_DT, tag="ident")
            from concourse.masks import make_identity
            make_identity(nc, ident[:])

            QT = persist.tile([P, SQ], MM_DT, tag="QT")
            KT = persist.tile([P, SQ], MM_DT, tag="KT")
            VA = persist.tile([P, SQ // P, 256], MM_DT, tag="VA")
            XT = persist.tile([P, SQ], MM_DT, tag="XT")

            # ones columns for the row-sum trick; per-chunk layout is
            # [ones 0:64 | A dims 64:128 | ones 128:192 | B dims 192:256]
            # (ones first so the sums land in PSUM partitions 0:64, which
            # reciprocal_approx_fast can read directly — the custom DVE op
            # mis-reads PSUM APs with a partition offset).
            # Only batch-0 chunks are needed before the first attention; the
            # rest is deferred into the ready queue.
            nc.vector.memset(VA[:, 0:16, 0:DK], 1.0)
            nc.vector.memset(VA[:, 0:16, P:P + DK], 1.0)

            # ---- filler queues ---------------------------------------
            # dq: projection units, gated on DMA safety (cur phase must be
            #     >= unit's enqueue phase + 2 so the input tile has landed).
            # rq: oproj units + memsets — data-resident, always safe.
            # need_p(tt) force-drains dq through tile tt (gate bypassed:
            # attention cannot start without its projections anyway).
            dq = deque()   # items: (tt, g_enq, thunk)
            rq = deque()   # items: (key, thunk)
            p_pending = {}
            ms_pending = [0]
            cur_g = [0]

            def dq_push(tt, g_enq, thunk):
                dq.append((tt, g_enq, thunk))
                p_pending[tt] = p_pending.get(tt, 0) + 1

            def dq_pop():
                tt, _, thunk = dq.popleft()
                p_pending[tt] -= 1
                thunk()

            def dq_safe():
                return bool(dq) and cur_g[0] >= dq[0][1] + 1

            def rq_pop():
                key, thunk = rq.popleft()
                if key == "ms":
                    ms_pending[0] -= 1
                thunk()

            def pump(n=1, prefer="d"):
                for _ in range(n):
                    if dq_safe():
                        dq_pop()
                    elif rq:
                        rq_pop()

            def need_p(tt):
                while p_pending.get(tt, 0) > 0:
                    if dq:
                        dq_pop()
                    elif rq:  # pragma: no cover - defensive
                        rq_pop()

            def need_ms():
                while ms_pending[0] > 0:
                    rq_pop()

            def pump_all():
                while dq or rq:
                    if dq:
                        dq_pop()
                    else:
                        rq_pop()

            # ---- projection ------------------------------------------
            def enqueue_proj(tt, g_enq):
                """Queue Q/K/V projections for token tile tt (512 tokens).

                DMAs are issued immediately (at enqueue time) so the data is
                resident by the time the matmul units are pumped.
                """
                if tt == 0:
                    qin, kin, vin = tile0["qin"], tile0["kin"], tile0["vin"]
                else:
                    qin = stream.tile([P, KO, 512], MM_DT, tag="qin", name="qin")
                    kin = stream.tile([P, KO, 512], MM_DT, tag="kin", name="kin")
                    vin = stream.tile([P, KO, 512], MM_DT, tag="vin", name="vin")
                    nc.sync.dma_start(qin[:], qT_d.ap()[tt])
                    nc.sync.dma_start(kin[:], kT_d.ap()[tt])
                    nc.sync.dma_start(vin[:], vT_d.ap()[tt])
                cols = bass.ts(tt, 512)
                state = {}

                def mk_proj(xin, w_t, dst_cols):
                    def half0():
                        ps = pp.tile([P, 512], F32, tag="pp", name="ps")
                        state["ps"] = ps
                        for ko in range(KO // 2):
                            nc.tensor.matmul(ps[:], w_t[:, ko, :],
                                             xin[:, ko, :],
                                             start=(ko == 0), stop=False)

                    def half1():
                        ps = state["ps"]
                        for ko in range(KO // 2, KO):
                            nc.tensor.matmul(ps[:], w_t[:, ko, :],
                                             xin[:, ko, :],
                                             start=False, stop=(ko == KO - 1))
                        nc.vector.tensor_copy(dst_cols, ps[:])
                    return half0, half1

                q0, q1 = mk_proj(qin, wq_t, QT[:, cols])
                k0, k1 = mk_proj(kin, wk_t, KT[:, cols])
                vts = vstage.tile([P, 512], MM_DT, tag="vts", name="vts")
                v0, v1 = mk_proj(vin, wv_t, vts[:])

                # one transpose+copy per unit: a monolithic 4x unit parks
                # DVE copies at the queue head waiting on PE transposes that
                # sit behind queued attention matmuls (head-of-line blocking
                # that stalls unrelated DVE work)
                def mk_vtrans(sub):
                    def vtrans():
                        tp = pp.tile([P, P], MM_DT, tag="pp", name="tp")
                        nc.tensor.transpose(tp[:], vts[:, bass.ts(sub, P)],
                                            ident[:])
                        kc = tt * 4 + sub
                        nc.vector.tensor_copy(
                            VA[:, kc].rearrange("p (a x) -> p a x", a=2)[:, :, DK:P],
                            tp[:].rearrange("p (a x) -> p a x", a=2))
                    return vtrans

                for u in (q0, q1, k0, k1, v0, v1,
                          mk_vtrans(0), mk_vtrans(1), mk_vtrans(2),
                          mk_vtrans(3)):
                    dq_push(tt, g_enq, u)

            # ---- output projection -----------------------------------
            def enqueue_oproj(tt):
                """Queue the output projection of q-tile tt: all 8
                output-dim chunks, contraction over this core's 128 x-dims,
                staged into one SBUF tile and shipped with one DMA."""
                ost = ostage.tile([P, KO, 512], F16, tag="ost", name="ost")

                def mk_unit(mo0):
                    def unit():
                        for mo in (mo0, mo0 + 1):
                            pso = pp.tile([P, 512], F32, tag="pp", name="pso")
                            nc.tensor.matmul(pso[:], wo_t[:, mo, :],
                                             XT[:, bass.ts(tt, 512)],
                                             start=True, stop=True)
                            if tt == NT - 1 and mo % 2 == 1:
                                # final tile: ACT is idle in the tail and the
                                # pure-DVE cast chain (8x690ns serial) was
                                # the tail's critical path — alternate, and
                                # ship quarters as they complete
                                nc.scalar.copy(ost[:, mo, :], pso[:])
                                q_dst = out_d.ap()[tt][:, mo - 1:mo + 1, :]
                                q_src = ost[:, mo - 1:mo + 1, :]
                                if mo % 4 == 1:
                                    nc.scalar.dma_start(q_dst, q_src)
                                else:
                                    nc.sync.dma_start(q_dst, q_src)
                            else:
                                nc.vector.tensor_copy(ost[:, mo, :], pso[:])
                                if mo == KO - 1:
                                    nc.sync.dma_start(out_d.ap()[tt], ost[:])
                    return unit

                for mo0 in (0, 2, 4, 6):
                    rq.append((("o", tt), mk_unit(mo0)))

            # ---- attention -------------------------------------------
            def attention(b, qt):
                """One 512-query tile of causal attention, both heads.

                Software-pipelined three chunks deep: the attn@V matmuls of
                chunk kc are emitted after the scores matmuls of chunk kc+3,
                so the ScalarE exp (+ GpSimd mask) of chunk kc has three full
                chunk periods of latency slack, and filler units are pumped
                per chunk to keep the PE queue dense.
                """
                qcols = bass.ds(b * S + qt * 512, 512)
                nkc = 4 * qt + 4
                ops = opp.tile([P, 2, 512], F32, tag="op", name="ops")
                pipe = deque()
                # dq first whenever its data has landed: prompt projection
                # consumption recycles stream-pool slots, which is what keeps
                # the input DMA pipeline moving. rq fills genuine droughts.
                prefer = "d"

                def scores_exp(kc):
                    kcols = bass.ds(b * S + kc * P, P)
                    j = kc - 4 * qt
                    co = max(j, 0) * P  # valid q-columns start here (causal)
                    w = 512 - co
                    ssc = scp.tile([P, 2, 512], F32, tag="sc", name="ssc")
                    for h in range(2):
                        rb = h * DK
                        nc.tensor.matmul(
                            ssc[:, h, co:],
                            KT[rb:rb + DK, kcols],
                            QT[rb:rb + DK, bass.ds(b * S + qt * 512 + co, w)],
                            start=True, stop=True)
                    e_t = epool.tile([P, 2, 512], MM_DT, tag="e", name="e_t")
                    nc.scalar.activation(e_t[:, :, co:], ssc[:, :, co:],
                                         mybir.ActivationFunctionType.Exp,
                                         scale=0.125)
                    if j >= 0:
                        # one op for both heads: mask broadcast over the
                        # head axis (halves the GpSimd latency in the
                        # exp -> mask -> attn chain on diagonal chunks)
                        nc.gpsimd.tensor_mul(
                            e_t[:, :, co:co + P], e_t[:, :, co:co + P],
                            mk_t[:].unsqueeze(1).broadcast_to((P, 2, P)))
                    pipe.append((e_t, co))

                def attn_mm(kc):
                    e_t, co = pipe.popleft()
                    gkc = b * (S // P) + kc
                    for h in range(2):
                        nc.tensor.matmul(ops[:, h, co:],
                                         VA[:, gkc, bass.ts(h, P)],
                                         e_t[:, h, co:],
                                         start=(kc == 0), stop=(kc == nkc - 1))

                rate = 2 if nkc <= 8 else 1
                scores_exp(0)
                scores_exp(1)
                pump(1, prefer)  # filler before sc(2)'s PSUM-slot reuse
                scores_exp(2)
                pump(rate, prefer)
                for kc in range(3, nkc):
                    scores_exp(kc)
                    attn_mm(kc - 3)
                    pump(rate, prefer)
                attn_mm(nkc - 3)
                pump(rate, prefer)
                attn_mm(nkc - 2)
                pump(1, prefer)
                attn_mm(nkc - 1)
                pump(2, prefer)  # keep the PE fed while the epilogue frees ops

                # normalize: sums are replicated in rows 0:64 of each head
                # (ones-first VA layout), attn values in rows 64:128
                r_t = rpool.tile([DK, 2, 512], F32, tag="r", name="r_t")
                nc.vector.reciprocal_approx_fast(r_t[:], ops[0:DK, :, :])
                for h in range(2):
                    nc.vector.tensor_mul(XT[h * DK:(h + 1) * DK, qcols],
                                         ops[DK:P, h, :], r_t[:, h, :])

            # ---- schedule --------------------------------------------
            for tt in range(4):
                enqueue_proj(tt, tt - 4)
            rq.append(("ms", lambda: nc.vector.memset(VA[:, 16:64, 0:DK], 1.0)))
            rq.append(("ms",
                       lambda: nc.vector.memset(VA[:, 16:64, P:P + DK], 1.0)))
            ms_pending[0] = 2
            for b in range(B):
                for qt in range(4):
                    g = 4 * b + qt
                    cur_g[0] = g
                    if b + 1 < B:
                        enqueue_proj(4 * (b + 1) + qt, g)
                    # oproj runs 4 tiles behind (XT persists), with catch-up
                    # at g=14/15: batch 3 has no next-batch projections to
                    # pump, so the backlog is what keeps its attention
                    # phases dense enough that HAM never re-throttles
                    if 4 <= g <= 13:
                        enqueue_oproj(g - 4)
                    elif g == 14:
                        enqueue_oproj(10)
                        enqueue_oproj(11)
                    elif g == 15:
                        enqueue_oproj(12)
                        enqueue_oproj(13)
                        enqueue_oproj(14)
                    for t in range(qt + 1):
                        need_p(4 * b + t)
                    if b > 0:
                        need_ms()
                    attention(b, qt)
            enqueue_oproj(NT - 1)
            pump_all()

    nc.compile()
    return nc


def _get_nc():
    global _NC
    if _NC is None:
        _NC = _build()
    return _NC


def _to_tiled_T(x2):
    """[SQ, D] fp32 -> [NT, 128, KO, 512] bf16 with x[g, d] at
    [g//512, d%128, d//128, g%512]."""
    xh = x2.astype(_np_mm_dt())
    return np.ascontiguousarray(
        xh.reshape(NT, 512, KO, P).transpose(0, 3, 2, 1))


def _weight_T(w_slice):
    """[128 out, 1024 in] -> [128 p, KO, 128 m] bf16 with W[m, d] at
    [d%128, d//128, m]."""
    return np.ascontiguousarray(
        w_slice.T.reshape(KO, P, P).transpose(1, 0, 2)).astype(_np_mm_dt())


def kernel(q, k, v, mask, W_q, W_k, W_v, W_o, b_o):
    global LAST_RESULT
    nc = _get_nc()

    qT = _to_tiled_T(np.asarray(q, np.float32).reshape(SQ, D))
    kT = _to_tiled_T(np.asarray(k, np.float32).reshape(SQ, D))
    vT = _to_tiled_T(np.asarray(v, np.float32).reshape(SQ, D))

    p_idx = np.arange(P)[:, None]
    f_idx = np.arange(P)[None, :]
    masks = (f_idx >= p_idx).astype(_np_mm_dt())

    W_q = np.asarray(W_q, np.float32)
    W_k = np.asarray(W_k, np.float32)
    W_v = np.asarray(W_v, np.float32)
    W_o = np.asarray(W_o, np.float32)

    in_maps = []
    for c in range(NCORES):
        cs = slice(c * P, (c + 1) * P)
        # wo: [k, mo, m] = W_o[mo*128+m, c*128+k]
        wo_c = np.ascontiguousarray(
            W_o[:, cs].reshape(KO, P, P).transpose(2, 0, 1)
        ).astype(_np_mm_dt())
        w_all = np.ascontiguousarray(np.stack(
            [_weight_T(W_q[cs, :]), _weight_T(W_k[cs, :]),
             _weight_T(W_v[cs, :]), wo_c], axis=1))
        in_maps.append({"qT": qT, "kT": kT, "vT": vT, "masks": masks,
                        "w": w_all})

    res = run_bass_kernel_spmd(nc, in_maps, core_ids=list(range(NCORES)),
                               trace=TRACE)
    LAST_RESULT = res

    acc = np.zeros((SQ, D), np.float32)
    for c in range(NCORES):
        # out: [NT, P, KO, 512] -> [KO*P, NT*512] = [D, SQ]
        partial_T = res.results[c]["out"].transpose(2, 1, 0, 3).reshape(D, SQ)
        acc += partial_T.T.astype(np.float32)
    acc += np.asarray(b_o, np.float32)
    return acc.reshape(B, S, D)


# revision 38
# speedup vs baseline: 1.0469x; 1.0225x over previous
"""Multi-head attention (B=4, S=2048, D=1024, H=16) on 8 Trainium2 NeuronCores.

Sharding: tensor-parallel over heads. Core c owns heads 2c, 2c+1 (a 128-wide
slice of the model dim). Each core computes Q/K/V projections for its head
slice over all tokens, causal attention for its 2 heads, and a partial output
projection (contraction over its 128 x-dims). The host sums the 8 partial
outputs and adds b_o.

All matmuls run in bf16 (full PE rate) with fp32 PSUM accumulation; softmax
runs without max-subtraction (scores are O(10), exp stays in range).

On-device layouts (T = transposed, tokens on the free axis):
  QT/KT: [128 head-dims, 8192 tokens] bf16 in SBUF
  VA:    [128 token-chunk, 64 chunks, 256] bf16; per-head cols = 64 ones
         followed by 64 V dims (the ones give replicated softmax row-sums
         for free, landing in PSUM partitions 0:64)
  Scores are computed transposed, S.T = [k-tokens, q-tokens], so softmax
  normalization lands on the free axis after the attn@V matmul.

Scheduling: both heads' score tiles share one 2-bank PSUM tile so a single
ACTIVATE does exp for both; the two heads' K=64 score matmuls row-tile into
the PE concurrently. Projection (next batch) and output-projection (lagged)
matmuls interleave between attention chunks through two filler queues:
  dq — DMA-dependent projection units, only pumped >= 2 phases after their
       input DMA was issued (a DMA-waiting matmul stalls the in-order PE
       queue, so unready units must never be pumped);
  rq — always-ready output-projection units (XT is resident) + memsets,
       preferred at batch transitions where dq's data is still in flight.
This keeps PE duty high everywhere so the HAM activity monitor never
re-throttles the PE clock to 1.2 GHz mid-kernel.

Output staging: all 8 output-dim chunks of a q-tile collect into one SBUF
tile and ship with a single DMA (16 issues total instead of 128), keeping
the Sync queue free for input streaming.
"""

import sys
import types
from collections import deque

sys.path.insert(0, "/opt/trn_rl_repo")

import numpy as np

# Optional: make run_bass_kernel_spmd(trace=True) work on images whose antenv
# lacks axon_hooks. Harmless if unavailable; kernel() defaults to trace=False.
try:  # pragma: no cover
    import antenv
    if "antenv.axon_hooks" not in sys.modules:
        from trn_agent_boot.trn_boot import _ntff_profile_via_ctypes

        _hook = _ntff_profile_via_ctypes("/opt/axon/libaxon_pjrt.so")
        _mod = types.ModuleType("antenv.axon_hooks")
        _mod.get_axon_ntff_profile_hook = lambda: _hook
        _mod.set_axon_ntff_profile_hook = lambda h: None
        sys.modules["antenv.axon_hooks"] = _mod
        antenv.axon_hooks = _mod
except Exception:
    pass

import concourse.bass as bass
import concourse.bacc as bacc
import concourse.tile as tile
import concourse.mybir as mybir
from concourse.bass_utils import run_bass_kernel_spmd

B, S, D, H = 4, 2048, 1024, 16
DK = D // H          # 64
P = 128
SQ = B * S           # 8192 tokens
NT = SQ // 512       # 16 token tiles of 512
KO = D // P          # 8 contraction chunks
NCORES = 8
F16 = mybir.dt.float16
F32 = mybir.dt.float32

TRACE = False        # set by test.py to capture an NTFF profile
LAST_RESULT = None   # BassKernelResults of the most recent run

MM_DT = mybir.dt.bfloat16
ACT_OPROJ_CAST = False  # all oproj casts on DVE: the ACT queue is the
                        # attention-phase rate cap (exp), keep it pure

_NC = None


def _np_mm_dt():
    if MM_DT == mybir.dt.float16:
        return np.float16
    import ml_dtypes
    return ml_dtypes.bfloat16


def _build():
    nc = bacc.Bacc("TRN2", target_bir_lowering=False, debug=False,
                   num_devices=NCORES)

    qT_d = nc.dram_tensor("qT", [NT, P, KO, 512], MM_DT, kind="ExternalInput")
    kT_d = nc.dram_tensor("kT", [NT, P, KO, 512], MM_DT, kind="ExternalInput")
    vT_d = nc.dram_tensor("vT", [NT, P, KO, 512], MM_DT, kind="ExternalInput")
    # all four weight matrices packed into one tensor: DMA engines are
    # packet-rate-bound (~50ns per per-partition segment), so one 8KB/
    # partition descriptor moves 4x the bytes of four 2KB/partition ones
    w_d = nc.dram_tensor("w", [P, 4, KO, P], MM_DT, kind="ExternalInput")
    mk_d = nc.dram_tensor("masks", [P, P], MM_DT, kind="ExternalInput")
    # [tt, p, mo, 512]: matches the SBUF staging tile order so one DMA per
    # q-tile ships all 8 output-dim chunks.
    out_d = nc.dram_tensor("out", [NT, P, KO, 512], F16, kind="ExternalOutput")

    with tile.TileContext(nc) as tc:
        with (
            tc.tile_pool(name="const", bufs=1) as const,
            tc.tile_pool(name="persist", bufs=1) as persist,
            tc.tile_pool(name="stream", bufs=3) as stream,
            tc.tile_pool(name="epool", bufs=6) as epool,
            tc.tile_pool(name="rpool", bufs=2) as rpool,
            tc.tile_pool(name="ostage", bufs=2) as ostage,
            tc.tile_pool(name="vstage", bufs=2) as vstage,
            tc.tile_pool(name="pp", bufs=2, space="PSUM") as pp,
            tc.tile_pool(name="scp", bufs=2, space="PSUM") as scp,
            tc.tile_pool(name="opp", bufs=1, space="PSUM") as opp,
        ):
            w_t = const.tile([P, 4, KO, P], MM_DT, tag="w")
            wq_t, wk_t, wv_t, wo_t = (w_t[:, j] for j in range(4))
            mk_t = const.tile([P, P], MM_DT, tag="mk")

            # Initial loads: few LARGE descriptors spread over the three
            # DMA-capable queues. Per-queue service is packet-rate-bound, so
            # splitting a tile into small segments serializes ~5us apiece on
            # one queue (measured) — full-tile 8KB/partition descriptors are
            # ~3x faster per byte. q on Sync, k on GpSimd, v split across
            # both behind them, weights (one descriptor) + mask on Scalar.
            tile0 = {}
            for nm, src in (("qin", qT_d), ("kin", kT_d), ("vin", vT_d)):
                tile0[nm] = stream.tile([P, KO, 512], MM_DT, tag=nm, name=nm)
            # ~1MB per queue, ordered by first consumer (q, then k, then v):
            # per-queue service rate is the startup bound, so balance bytes
            nc.sync.dma_start(tile0["qin"][:, 0:KO // 2, :],
                              qT_d.ap()[0, :, 0:KO // 2, :])
            nc.gpsimd.dma_start(tile0["qin"][:, KO // 2:, :],
                                qT_d.ap()[0, :, KO // 2:, :])
            nc.gpsimd.dma_start(tile0["kin"][:, 0:KO // 2, :],
                                kT_d.ap()[0, :, 0:KO // 2, :])
            nc.sync.dma_start(tile0["kin"][:, KO // 2:, :],
                              kT_d.ap()[0, :, KO // 2:, :])
            nc.scalar.dma_start(w_t[:], w_d.ap())
            nc.scalar.dma_start(tile0["vin"][:, 0:KO // 2, :],
                                vT_d.ap()[0, :, 0:KO // 2, :])
            nc.gpsimd.dma_start(tile0["vin"][:, KO // 2:KO - 2, :],
                                vT_d.ap()[0, :, KO // 2:KO - 2, :])
            nc.sync.dma_start(tile0["vin"][:, KO - 2:, :],
                              vT_d.ap()[0, :, KO - 2:, :])
            nc.scalar.dma_start(mk_t[:], mk_d.ap())

            ident = const.tile([P, P], MM_DT, tag="ident")
            from concourse.masks import make_identity
            make_identity(nc, ident[:])

            QT = persist.tile([P, SQ], MM_DT, tag="QT")
            KT = persist.tile([P, SQ], MM_DT, tag="KT")
            VA = persist.tile([P, SQ // P, 256], MM_DT, tag="VA")
            XT = persist.tile([P, SQ], MM_DT, tag="XT")

            # ones columns for the row-sum trick; per-chunk layout is
            # [ones 0:64 | A dims 64:128 | ones 128:192 | B dims 192:256]
            # (ones first so the sums land in PSUM partitions 0:64, which
            # reciprocal_approx_fast can read directly — the custom DVE op
            # mis-reads PSUM APs with a partition offset).
            # Only batch-0 chunks are needed before the first attention; the
            # rest is deferred into the ready queue.
            nc.vector.memset(VA[:, 0:16, 0:DK], 1.0)
            nc.vector.memset(VA[:, 0:16, P:P + DK], 1.0)

            # ---- filler queues ---------------------------------------
            # dq: projection units, gated on DMA safety (cur phase must be
            #     >= unit's enqueue phase + 2 so the input tile has landed).
            # rq: oproj units + memsets — data-resident, always safe.
            # need_p(tt) force-drains dq through tile tt (gate bypassed:
            # attention cannot start without its projections anyway).
            dq = deque()   # items: (tt, g_enq, thunk)
            rq = deque()   # items: (key, thunk)
            p_pending = {}
            ms_pending = [0]
            cur_g = [0]

            def dq_push(tt, g_enq, thunk):
                dq.append((tt, g_enq, thunk))
                p_pending[tt] = p_pending.get(tt, 0) + 1

            def dq_pop():
                tt, _, thunk = dq.popleft()
                p_pending[tt] -= 1
                thunk()

            def dq_safe():
                return bool(dq) and cur_g[0] >= dq[0][1] + 1

            def rq_pop():
                key, thunk = rq.popleft()
                if key == "ms":
                    ms_pending[0] -= 1
                thunk()

            def pump(n=1, prefer="d"):
                for _ in range(n):
                    if dq_safe():
                        dq_pop()
                    elif rq:
                        rq_pop()

            def need_p(tt):
                while p_pending.get(tt, 0) > 0:
                    if dq:
                        dq_pop()
                    elif rq:  # pragma: no cover - defensive
                        rq_pop()

            def need_ms():
                while ms_pending[0] > 0:
                    rq_pop()

            def pump_all():
                while dq or rq:
                    if dq:
                        dq_pop()
                    else:
                        rq_pop()

            # ---- projection ------------------------------------------
            def enqueue_proj(tt, g_enq):
                """Queue Q/K/V projections for token tile tt (512 tokens).

                DMAs are issued immediately (at enqueue time) so the data is
                resident by the time the matmul units are pumped.
                """
                if tt == 0:
                    qin, kin, vin = tile0["qin"], tile0["kin"], tile0["vin"]
                else:
                    qin = stream.tile([P, KO, 512], MM_DT, tag="qin", name="qin")
                    kin = stream.tile([P, KO, 512], MM_DT, tag="kin", name="kin")
                    vin = stream.tile([P, KO, 512], MM_DT, tag="vin", name="vin")
                    nc.sync.dma_start(qin[:], qT_d.ap()[tt])
                    nc.sync.dma_start(kin[:], kT_d.ap()[tt])
                    nc.sync.dma_start(vin[:], vT_d.ap()[tt])
                cols = bass.ts(tt, 512)
                state = {}

                def mk_proj(xin, w_t, dst_cols):
                    def half0():
                        ps = pp.tile([P, 512], F32, tag="pp", name="ps")
                        state["ps"] = ps
                        for ko in range(KO // 2):
                            nc.tensor.matmul(ps[:], w_t[:, ko, :],
                                             xin[:, ko, :],
                                             start=(ko == 0), stop=False)

                    def half1():
                        ps = state["ps"]
                        for ko in range(KO // 2, KO):
                            nc.tensor.matmul(ps[:], w_t[:, ko, :],
                                             xin[:, ko, :],
                                             start=False, stop=(ko == KO - 1))
                        nc.vector.tensor_copy(dst_cols, ps[:])
                    return half0, half1

                q0, q1 = mk_proj(qin, wq_t, QT[:, cols])
                k0, k1 = mk_proj(kin, wk_t, KT[:, cols])
                vts = vstage.tile([P, 512], MM_DT, tag="vts", name="vts")
                v0, v1 = mk_proj(vin, wv_t, vts[:])

                # one transpose+copy per unit: a monolithic 4x unit parks
                # DVE copies at the queue head waiting on PE transposes that
                # sit behind queued attention matmuls (head-of-line blocking
                # that stalls unrelated DVE work)
                def mk_vtrans(sub):
                    def vtrans():
                        tp = pp.tile([P, P], MM_DT, tag="pp", name="tp")
                        nc.tensor.transpose(tp[:], vts[:, bass.ts(sub, P)],
                                            ident[:])
                        kc = tt * 4 + sub
                        nc.vector.tensor_copy(
                            VA[:, kc].rearrange("p (a x) -> p a x", a=2)[:, :, DK:P],
                            tp[:].rearrange("p (a x) -> p a x", a=2))
                    return vtrans

                for u in (q0, q1, k0, k1, v0, v1,
                          mk_vtrans(0), mk_vtrans(1), mk_vtrans(2),
                          mk_vtrans(3)):
                    dq_push(tt, g_enq, u)

            # ---- output projection -----------------------------------
            def enqueue_oproj(tt):
                """Queue the output projection of q-tile tt: all 8
                output-dim chunks, contraction over this core's 128 x-dims,
                staged into one SBUF tile and shipped with one DMA."""
                ost = ostage.tile([P, KO, 512], F16, tag="ost", name="ost")

                def mk_unit(mo0):
                    def unit():
                        for mo in (mo0, mo0 + 1):
                            pso = pp.tile([P, 512], F32, tag="pp", name="pso")
                            nc.tensor.matmul(pso[:], wo_t[:, mo, :],
                                             XT[:, bass.ts(tt, 512)],
                                             start=True, stop=True)
                            if tt == NT - 1 and mo % 2 == 1:
                                # final tile: ACT is idle in the tail and the
                                # pure-DVE cast chain (8x690ns serial) was
                                # the tail's critical path — alternate, and
                                # ship quarters as they complete
                                nc.scalar.copy(ost[:, mo, :], pso[:])
                                q_dst = out_d.ap()[tt][:, mo - 1:mo + 1, :]
                                q_src = ost[:, mo - 1:mo + 1, :]
                                if mo % 4 == 1:
                                    nc.scalar.dma_start(q_dst, q_src)
                                else:
                                    nc.sync.dma_start(q_dst, q_src)
                            else:
                                nc.vector.tensor_copy(ost[:, mo, :], pso[:])
                                if mo == KO - 1:
                                    nc.sync.dma_start(out_d.ap()[tt], ost[:])
                    return unit

                for mo0 in (0, 2, 4, 6):
                    rq.append((("o", tt), mk_unit(mo0)))

            # ---- attention -------------------------------------------
            def attention(b, qt):
                """One 512-query tile of causal attention, both heads.

                Software-pipelined three chunks deep: the attn@V matmuls of
                chunk kc are emitted after the scores matmuls of chunk kc+3,
                so the ScalarE exp (+ GpSimd mask) of chunk kc has three full
                chunk periods of latency slack, and filler units are pumped
                per chunk to keep the PE queue dense.
                """
                qcols = bass.ds(b * S + qt * 512, 512)
                nkc = 4 * qt + 4
                ops = opp.tile([P, 2, 512], F32, tag="op", name="ops")
                pipe = deque()
                # dq first whenever its data has landed: prompt projection
                # consumption recycles stream-pool slots, which is what keeps
                # the input DMA pipeline moving. rq fills genuine droughts.
                prefer = "d"

                def scores_exp(kc):
                    kcols = bass.ds(b * S + kc * P, P)
                    j = kc - 4 * qt
                    co = max(j, 0) * P  # valid q-columns start here (causal)
                    w = 512 - co
                    ssc = scp.tile([P, 2, 512], F32, tag="sc", name="ssc")
                    for h in range(2):
                        rb = h * DK
                        nc.tensor.matmul(
                            ssc[:, h, co:],
                            KT[rb:rb + DK, kcols],
                            QT[rb:rb + DK, bass.ds(b * S + qt * 512 + co, w)],
                            start=True, stop=True)
                    e_t = epool.tile([P, 2, 512], MM_DT, tag="e", name="e_t")
                    nc.scalar.activation(e_t[:, :, co:], ssc[:, :, co:],
                                         mybir.ActivationFunctionType.Exp,
                                         scale=0.125)
                    if j >= 0:
                        # one op for both heads: mask broadcast over the
                        # head axis (halves the GpSimd latency in the
                        # exp -> mask -> attn chain on diagonal chunks)
                        nc.gpsimd.tensor_mul(
                            e_t[:, :, co:co + P], e_t[:, :, co:co + P],
                            mk_t[:].unsqueeze(1).broadcast_to((P, 2, P)))
                    pipe.append((e_t, co))

                def attn_mm(kc):
                    e_t, co = pipe.popleft()
                    gkc = b * (S // P) + kc
                    for h in range(2):
                        nc.tensor.matmul(ops[:, h, co:],
                                         VA[:, gkc, bass.ts(h, P)],
                                         e_t[:, h, co:],
                                         start=(kc == 0), stop=(kc == nkc - 1))

                # depth-4 software pipeline: attn@V of chunk kc is emitted
                # after the scores of chunk kc+4. The extra chunk (vs 3)
                # moves this tile's first attn matmul — which reuses the opp
                # banks via start=True — far enough from the previous tile's
                # normalize muls that their PSUM-read latency is hidden.
                rate = 2 if nkc <= 8 else 1
                scores_exp(0)
                scores_exp(1)
                pump(1, prefer)  # filler before sc(2)'s PSUM-slot reuse
                scores_exp(2)
                pump(rate, prefer)
                scores_exp(3)
                pump(1, prefer)
                for kc in range(4, nkc):
                    scores_exp(kc)
                    attn_mm(kc - 4)
                    pump(rate, prefer)
                attn_mm(nkc - 4)
                pump(rate, prefer)
                attn_mm(nkc - 3)
                pump(1, prefer)
                attn_mm(nkc - 2)
                pump(1, prefer)
                attn_mm(nkc - 1)
                pump(2, prefer)  # keep the PE fed while the epilogue frees ops

                # normalize: sums are replicated in rows 0:64 of each head
                # (ones-first VA layout), attn values in rows 64:128
                r_t = rpool.tile([DK, 2, 512], F32, tag="r", name="r_t")
                nc.vector.reciprocal_approx_fast(r_t[:], ops[0:DK, :, :])
                for h in range(2):
                    nc.vector.tensor_mul(XT[h * DK:(h + 1) * DK, qcols],
                                         ops[DK:P, h, :], r_t[:, h, :])

            # ---- schedule --------------------------------------------
            for tt in range(4):
                enqueue_proj(tt, tt - 4)
            rq.append(("ms", lambda: nc.vector.memset(VA[:, 16:64, 0:DK], 1.0)))
            rq.append(("ms",
                       lambda: nc.vector.memset(VA[:, 16:64, P:P + DK], 1.0)))
            ms_pending[0] = 2
            for b in range(B):
                for qt in range(4):
                    g = 4 * b + qt
                    cur_g[0] = g
                    if b + 1 < B:
                        enqueue_proj(4 * (b + 1) + qt, g)
                    # oproj runs 4 tiles behind (XT persists), with catch-up
                    # at g=14/15: batch 3 has no next-batch projections to
                    # pump, so the backlog is what keeps its attention
                    # phases dense enough that HAM never re-throttles
                    if 4 <= g <= 13:
                        enqueue_oproj(g - 4)
                    elif g == 14:
                        enqueue_oproj(10)
                        enqueue_oproj(11)
                    elif g == 15:
                        enqueue_oproj(12)
                        enqueue_oproj(13)
                        enqueue_oproj(14)
                    for t in range(qt + 1):
                        need_p(4 * b + t)
                    if b > 0:
                        need_ms()
                    attention(b, qt)
            enqueue_oproj(NT - 1)
            pump_all()

    nc.compile()
    return nc


def _get_nc():
    global _NC
    if _NC is None:
        _NC = _build()
    return _NC


def _to_tiled_T(x2):
    """[SQ, D] fp32 -> [NT, 128, KO, 512] bf16 with x[g, d] at
    [g//512, d%128, d//128, g%512]."""
    xh = x2.astype(_np_mm_dt())
    return np.ascontiguousarray(
        xh.reshape(NT, 512, KO, P).transpose(0, 3, 2, 1))


def _weight_T(w_slice):
    """[128 out, 1024 in] -> [128 p, KO, 128 m] bf16 with W[m, d] at
    [d%128, d//128, m]."""
    return np.ascontiguousarray(
        w_slice.T.reshape(KO, P, P).transpose(1, 0, 2)).astype(_np_mm_dt())


def kernel(q, k, v, mask, W_q, W_k, W_v, W_o, b_o):
    global LAST_RESULT
    nc = _get_nc()

    qT = _to_tiled_T(np.asarray(q, np.float32).reshape(SQ, D))
    kT = _to_tiled_T(np.asarray(k, np.float32).reshape(SQ, D))
    vT = _to_tiled_T(np.asarray(v, np.float32).reshape(SQ, D))

    p_idx = np.arange(P)[:, None]
    f_idx = np.arange(P)[None, :]
    masks = (f_idx >= p_idx).astype(_np_mm_dt())

    W_q = np.asarray(W_q, np.float32)
    W_k = np.asarray(W_k, np.float32)
    W_v = np.asarray(W_v, np.float32)
    W_o = np.asarray(W_o, np.float32)

    in_maps = []
    for c in range(NCORES):
        cs = slice(c * P, (c + 1) * P)
        # wo: [k, mo, m] = W_o[mo*128+m, c*128+k]
        wo_c = np.ascontiguousarray(
            W_o[:, cs].reshape(KO, P, P).transpose(2, 0, 1)
        ).astype(_np_mm_dt())
        w_all = np.ascontiguousarray(np.stack(
            [_weight_T(W_q[cs, :]), _weight_T(W_k[cs, :]),
             _weight_T(W_v[cs, :]), wo_c], axis=1))
        in_maps.append({"qT": qT, "kT": kT, "vT": vT, "masks": masks,
                        "w": w_all})

    res = run_bass_kernel_spmd(nc, in_maps, core_ids=list(range(NCORES)),
                               trace=TRACE)
    LAST_RESULT = res

    acc = np.zeros((SQ, D), np.float32)
    for c in range(NCORES):
        # out: [NT, P, KO, 512] -> [KO*P, NT*512] = [D, SQ]
        partial_T = res.results[c]["out"].transpose(2, 1, 0, 3).reshape(D, SQ)
        acc += partial_T.T.astype(np.float32)
    acc += np.asarray(b_o, np.float32)
    return acc.reshape(B, S, D)
